# revision 7
# baseline (speedup 1.0000x reference)
"""GCN layer (gather-gate-sum / dense / gather-sum) on 8 Trainium2 NeuronCores.

Sharding: nodes are split across the 8 cores (2500 rows each, padded to 2560).
The full node-feature table (h, then h2) stays replicated in each core's DRAM
and the neighbor gather is a DMAGather against it, so no halo exchange is
needed inside a launch.  The round-1 -> round-2 dependency (every core needs
every h2 row) is satisfied by a host-side gather between two launches.

Self-contained: shapes are hardcoded for N=20000, D=32, F=128, 8 cores.
"""
import os
import sys

sys.path.insert(0, "/opt/trn_rl_repo")

import numpy as np

N_NODES = 20000
DEGREE = 32
F = 128
N_CORES = 8
ROWS_PER_CORE = N_NODES // N_CORES          # 2500
NBLK = (ROWS_PER_CORE + 127) // 128         # 20 blocks of 128 rows
ROWS_PAD = NBLK * 128                       # 2560
PAIRS_BLK = 128 * DEGREE                    # 4096 gather indices per block
IDXC = PAIRS_BLK // 16                      # idx columns per block (wrapped in 16)

_cache = {}


def _wrap_idx(idx_flat):
    """Pack linear gather indices into the [128, n/16] int16 SBUF layout
    (index i lives at partition i%16, column i//16; replicated to 128)."""
    n = idx_flat.shape[0]
    assert n % 16 == 0
    w = np.zeros((16, n // 16), dtype=np.int16)
    w[np.arange(n) % 16, np.arange(n) // 16] = idx_flat.astype(np.int16)
    return np.tile(w, (8, 1))


def _gather_idx_for_core(nbrs_shard):
    """nbrs_shard: [ROWS_PAD, DEGREE] int.  Block b gathers its 128 rows'
    neighbors with linear order i = d*128 + p  (partition p = row-in-block,
    free block d = neighbor slot); wrapped layout [16, n/16] replicated x8."""
    lin = nbrs_shard.reshape(NBLK, 128, DEGREE).transpose(0, 2, 1).reshape(NBLK, PAIRS_BLK)
    w = lin.reshape(NBLK, IDXC, 16).transpose(0, 2, 1).astype(np.int16)  # [b, 16, IDXC]
    w = w.transpose(1, 0, 2).reshape(16, NBLK * IDXC)
    return np.tile(w, (8, 1))


def _build_launch1():
    import concourse.bacc as bacc
    import concourse.mybir as mybir
    from concourse.mybir import AluOpType
    from concourse.tile import TileContext

    dt = mybir.dt
    nc = bacc.Bacc("TRN2", target_bir_lowering=False, debug=False)
    h32 = nc.dram_tensor("h32", [N_NODES, F], dt.float32, kind="ExternalInput")
    idx1 = nc.dram_tensor("idx1", [128, NBLK * IDXC], dt.int16, kind="ExternalInput")
    wg = nc.dram_tensor("wg", [ROWS_PAD, F], dt.float32, kind="ExternalInput")
    bg = nc.dram_tensor("bg", [ROWS_PAD, 1], dt.float32, kind="ExternalInput")
    nm = nc.dram_tensor("nm", [ROWS_PAD, 1], dt.float32, kind="ExternalInput")
    wei = nc.dram_tensor("wei", [F, F], dt.float32, kind="ExternalInput")
    ident = nc.dram_tensor("ident", [128, 128], dt.float32, kind="ExternalInput")
    h2o = nc.dram_tensor("h2o", [ROWS_PAD, F], dt.float32, kind="ExternalOutput")

    wg_r = wg.ap().rearrange("(b p) f -> b p f", p=128)
    bg_r = bg.ap().rearrange("(b p) o -> b p o", p=128)
    nm_r = nm.ap().rearrange("(b p) o -> b p o", p=128)
    h2o_r = h2o.ap().rearrange("(b p) f -> b p f", p=128)

    with TileContext(nc) as tc:
        with (
            tc.tile_pool(name="const", bufs=1) as cpool,
            tc.tile_pool(name="mail", bufs=3) as mpool,
            tc.tile_pool(name="tmp", bufs=3) as tpool,
            tc.tile_pool(name="small", bufs=4) as spool,
            tc.tile_pool(name="out", bufs=3) as opool,
            tc.tile_pool(name="ps", bufs=4, space="PSUM") as pspool,
        ):
            idx_sb = cpool.tile([128, NBLK * IDXC], dt.int16)
            nc.sync.dma_start(idx_sb[:], idx1.ap())
            wei_sb = cpool.tile([F, F], dt.float32)
            nc.sync.dma_start(wei_sb[:], wei.ap())
            id_sb = cpool.tile([128, 128], dt.float32)
            nc.sync.dma_start(id_sb[:], ident.ap())

            for b in range(NBLK):
                wg_t = spool.tile([128, F], dt.float32, tag="wg")
                nc.sync.dma_start(wg_t[:], wg_r[b])
                bg_t = spool.tile([128, 1], dt.float32, tag="bg")
                nc.sync.dma_start(bg_t[:], bg_r[b])
                nm_t = spool.tile([128, 1], dt.float32, tag="nm")
                nc.sync.dma_start(nm_t[:], nm_r[b])

                mail = mpool.tile([128, PAIRS_BLK], dt.float32)
                nc.gpsimd.dma_gather(
                    mail[:].rearrange("p (c f) -> p c f", f=F),
                    h32.ap(), idx_sb[:, b * IDXC:(b + 1) * IDXC],
                    PAIRS_BLK, PAIRS_BLK, F, single_packet=False,
                )
                m3 = mail[:].rearrange("p (d f) -> p d f", d=DEGREE)

                # logits[p, d] = sum_f mail[p, d, f] * wg[p, f]
                tmp = tpool.tile([128, PAIRS_BLK], dt.float32)
                wg_b = wg_t[:].unsqueeze(1).broadcast_to([128, DEGREE, F])
                nc.vector.tensor_tensor(
                    tmp[:].rearrange("p (d f) -> p d f", d=DEGREE),
                    m3, wg_b, AluOpType.mult,
                )
                lg = spool.tile([128, DEGREE], dt.float32, tag="lg")
                nc.vector.reduce_sum(
                    lg[:], tmp[:].rearrange("p (d f) -> p d f", d=DEGREE),
                    axis=mybir.AxisListType.X,
                )
                # mask = (logits + b_gate) > 0   (== round(sigmoid(.)))
                nc.vector.tensor_scalar(lg[:], lg[:], bg_t[:], None, AluOpType.add)
                mk = spool.tile([128, DEGREE], dt.float32, tag="mk")
                nc.vector.tensor_scalar(mk[:], lg[:], 0.0, None, AluOpType.is_gt)

                # h1 = sum_d mask * mail   (masked mult, then d-halving tree)
                mk_b = mk[:].unsqueeze(2).broadcast_to([128, DEGREE, F])
                nc.vector.tensor_tensor(
                    tmp[:].rearrange("p (d f) -> p d f", d=DEGREE),
                    m3, mk_b, AluOpType.mult,
                )
                w = PAIRS_BLK // 2
                while w >= F:
                    nc.vector.tensor_tensor(
                        tmp[:, :w], tmp[:, :w], tmp[:, w:2 * w], AluOpType.add,
                    )
                    w //= 2
                # h1 *= norm
                nc.vector.tensor_scalar(
                    tmp[:, :F], tmp[:, :F], nm_t[:], None, AluOpType.mult,
                )
                # h2 = h1 @ weight  (transpose h1 on PE, then matmul)
                h1T_ps = pspool.tile([128, 128], dt.float32, tag="tp")
                nc.tensor.transpose(h1T_ps[:], tmp[:, :F], id_sb[:])
                h1T = opool.tile([128, 128], dt.float32, tag="h1T")
                nc.vector.tensor_copy(h1T[:], h1T_ps[:])
                h2_ps = pspool.tile([128, F], dt.float32, tag="mm")
                nc.tensor.matmul(h2_ps[:], h1T[:], wei_sb[:], start=True, stop=True)
                h2_sb = opool.tile([128, F], dt.float32, tag="h2")
                nc.vector.tensor_copy(h2_sb[:], h2_ps[:])
                nc.sync.dma_start(h2o_r[b], h2_sb[:])
    nc.finalize()
    return nc


def _build_launch2():
    import concourse.bacc as bacc
    import concourse.mybir as mybir
    from concourse.mybir import AluOpType
    from concourse.tile import TileContext

    dt = mybir.dt
    nc = bacc.Bacc("TRN2", target_bir_lowering=False, debug=False)
    h2f = nc.dram_tensor("h2f", [N_NODES, F], dt.float32, kind="ExternalInput")
    idx2 = nc.dram_tensor("idx2", [128, NBLK * IDXC], dt.int16, kind="ExternalInput")
    nm = nc.dram_tensor("nm", [ROWS_PAD, 1], dt.float32, kind="ExternalInput")
    bia = nc.dram_tensor("bia", [128, F], dt.float32, kind="ExternalInput")
    h3o = nc.dram_tensor("h3o", [ROWS_PAD, F], dt.float32, kind="ExternalOutput")

    nm_r = nm.ap().rearrange("(b p) o -> b p o", p=128)
    h3o_r = h3o.ap().rearrange("(b p) f -> b p f", p=128)

    with TileContext(nc) as tc:
        with (
            tc.tile_pool(name="const", bufs=1) as cpool,
            tc.tile_pool(name="mail", bufs=4) as mpool,
            tc.tile_pool(name="small", bufs=4) as spool,
            tc.tile_pool(name="out", bufs=3) as opool,
        ):
            idx_sb = cpool.tile([128, NBLK * IDXC], dt.int16)
            nc.sync.dma_start(idx_sb[:], idx2.ap())
            bia_sb = cpool.tile([128, F], dt.float32)
            nc.sync.dma_start(bia_sb[:], bia.ap())

            for b in range(NBLK):
                nm_t = spool.tile([128, 1], dt.float32, tag="nm")
                nc.sync.dma_start(nm_t[:], nm_r[b])
                g = mpool.tile([128, PAIRS_BLK], dt.float32)
                nc.gpsimd.dma_gather(
                    g[:].rearrange("p (c f) -> p c f", f=F),
                    h2f.ap(), idx_sb[:, b * IDXC:(b + 1) * IDXC],
                    PAIRS_BLK, PAIRS_BLK, F, single_packet=False,
                )
                w = PAIRS_BLK // 2
                while w >= F:
                    nc.vector.tensor_tensor(
                        g[:, :w], g[:, :w], g[:, w:2 * w], AluOpType.add,
                    )
                    w //= 2
                nc.vector.tensor_scalar(
                    g[:, :F], g[:, :F], nm_t[:], None, AluOpType.mult,
                )
                h3 = opool.tile([128, F], dt.float32, tag="h3")
                nc.vector.tensor_tensor(h3[:], g[:, :F], bia_sb[:], AluOpType.add)
                nc.vector.tensor_scalar(h3[:], h3[:], 0.0, None, AluOpType.max)
                nc.sync.dma_start(h3o_r[b], h3[:])
    nc.finalize()
    return nc


def _get(name, builder):
    if name not in _cache:
        _cache[name] = builder()
    return _cache[name]


def kernel(h, neighbors, norm, W_gate, b_gate, weight, bias):
    from concourse import bass_utils

    h = np.asarray(h, dtype=np.float32)
    neighbors_in = np.asarray(neighbors)
    neighbors = neighbors_in.astype(np.int64)
    norm = np.asarray(norm, dtype=np.float32).reshape(N_NODES, 1)
    W_gate = np.asarray(W_gate, dtype=np.float32)
    b_gate = np.asarray(b_gate, dtype=np.float32).reshape(N_NODES, 1)
    weight = np.asarray(weight, dtype=np.float32)
    bias = np.asarray(bias, dtype=np.float32)

    pad = ROWS_PAD - ROWS_PER_CORE
    ident = np.eye(128, dtype=np.float32)
    bias_bc = np.broadcast_to(bias, (128, F)).copy()

    nc1 = _get("l1", _build_launch1)
    in_maps1 = []
    for c in range(N_CORES):
        s = slice(c * ROWS_PER_CORE, (c + 1) * ROWS_PER_CORE)
        nb = np.concatenate([neighbors[s], np.zeros((pad, DEGREE), np.int64)])
        in_maps1.append({
            "h32": h,
            "idx1": _gather_idx_for_core(nb),
            "wg": np.concatenate([W_gate[s], np.zeros((pad, F), np.float32)]),
            "bg": np.concatenate([b_gate[s], np.zeros((pad, 1), np.float32)]),
            "nm": np.concatenate([norm[s], np.zeros((pad, 1), np.float32)]),
            "wei": weight,
            "ident": ident,
        })
    import time as _time
    _t0 = _time.perf_counter()
    res1 = bass_utils.run_bass_kernel_spmd(nc1, in_maps1, core_ids=list(range(N_CORES)))
    _t1 = _time.perf_counter()
    kernel.launch_times = [_t1 - _t0]
    h2 = np.concatenate(
        [res1.results[c]["h2o"][:ROWS_PER_CORE] for c in range(N_CORES)]
    )

    nc2 = _get("l2", _build_launch2)
    in_maps2 = []
    for c in range(N_CORES):
        s = slice(c * ROWS_PER_CORE, (c + 1) * ROWS_PER_CORE)
        nb = np.concatenate([neighbors[s], np.zeros((pad, DEGREE), np.int64)])
        in_maps2.append({
            "h2f": h2,
            "idx2": _gather_idx_for_core(nb),
            "nm": np.concatenate([norm[s], np.zeros((pad, 1), np.float32)]),
            "bia": bias_bc,
        })
    _t0 = _time.perf_counter()
    res2 = bass_utils.run_bass_kernel_spmd(nc2, in_maps2, core_ids=list(range(N_CORES)))
    _t1 = _time.perf_counter()
    kernel.launch_times.append(_t1 - _t0)
    out = np.concatenate(
        [res2.results[c]["h3o"][:ROWS_PER_CORE] for c in range(N_CORES)]
    )
    return out.astype(np.float32)


# revision 8
# speedup vs baseline: 1.0331x; 1.0331x over previous
"""GCN layer (gather-gate-sum / dense / gather-sum) on 8 Trainium2 NeuronCores.

Sharding: nodes are split across the 8 cores (2500 rows each, padded to 2560).
The full node-feature table (h, then h2) stays replicated in each core's DRAM
and the neighbor gather is a DMAGather against it, so no halo exchange is
needed inside a launch.  The round-1 -> round-2 dependency (every core needs
every h2 row) is satisfied by a host-side gather between two launches.

Self-contained: shapes are hardcoded for N=20000, D=32, F=128, 8 cores.
"""
import os
import sys

sys.path.insert(0, "/opt/trn_rl_repo")

import numpy as np

N_NODES = 20000
DEGREE = 32
F = 128
N_CORES = 8
ROWS_PER_CORE = N_NODES // N_CORES          # 2500
NBLK = (ROWS_PER_CORE + 127) // 128         # 20 blocks of 128 rows
ROWS_PAD = NBLK * 128                       # 2560
PAIRS_BLK = 128 * DEGREE                    # 4096 gather indices per block
IDXC = PAIRS_BLK // 16                      # idx columns per block (wrapped in 16)

_cache = {}


def _wrap_idx(idx_flat):
    """Pack linear gather indices into the [128, n/16] int16 SBUF layout
    (index i lives at partition i%16, column i//16; replicated to 128)."""
    n = idx_flat.shape[0]
    assert n % 16 == 0
    w = np.zeros((16, n // 16), dtype=np.int16)
    w[np.arange(n) % 16, np.arange(n) // 16] = idx_flat.astype(np.int16)
    return np.tile(w, (8, 1))


def _gather_idx_for_core(nbrs_shard):
    """nbrs_shard: [ROWS_PAD, DEGREE] int.  Block b gathers its 128 rows'
    neighbors with linear order i = d*128 + p  (partition p = row-in-block,
    free block d = neighbor slot); wrapped layout [16, n/16] replicated x8."""
    lin = nbrs_shard.reshape(NBLK, 128, DEGREE).transpose(0, 2, 1).reshape(NBLK, PAIRS_BLK)
    w = lin.reshape(NBLK, IDXC, 16).transpose(0, 2, 1).astype(np.int16)  # [b, 16, IDXC]
    w = w.transpose(1, 0, 2).reshape(16, NBLK * IDXC)
    return np.tile(w, (8, 1))


def _build_launch1():
    import concourse.bacc as bacc
    import concourse.mybir as mybir
    from concourse.mybir import AluOpType
    from concourse.tile import TileContext

    dt = mybir.dt
    nc = bacc.Bacc("TRN2", target_bir_lowering=False, debug=False)
    h32 = nc.dram_tensor("h32", [N_NODES, F], dt.float32, kind="ExternalInput")
    idx1 = nc.dram_tensor("idx1", [128, NBLK * IDXC], dt.int16, kind="ExternalInput")
    wg = nc.dram_tensor("wg", [ROWS_PAD, F], dt.float32, kind="ExternalInput")
    bg = nc.dram_tensor("bg", [ROWS_PAD, 1], dt.float32, kind="ExternalInput")
    nm = nc.dram_tensor("nm", [ROWS_PAD, 1], dt.float32, kind="ExternalInput")
    wei = nc.dram_tensor("wei", [F, F], dt.float32, kind="ExternalInput")
    ident = nc.dram_tensor("ident", [128, 128], dt.float32, kind="ExternalInput")
    h2o = nc.dram_tensor("h2o", [ROWS_PAD, F], dt.float32, kind="ExternalOutput")

    wg_r = wg.ap().rearrange("(b p) f -> b p f", p=128)
    bg_r = bg.ap().rearrange("(b p) o -> b p o", p=128)
    nm_r = nm.ap().rearrange("(b p) o -> b p o", p=128)
    h2o_r = h2o.ap().rearrange("(b p) f -> b p f", p=128)

    with TileContext(nc) as tc:
        with (
            tc.tile_pool(name="const", bufs=1) as cpool,
            tc.tile_pool(name="mail", bufs=3) as mpool,
            tc.tile_pool(name="tmp", bufs=3) as tpool,
            tc.tile_pool(name="small", bufs=4) as spool,
            tc.tile_pool(name="out", bufs=3) as opool,
            tc.tile_pool(name="ps", bufs=4, space="PSUM") as pspool,
        ):
            idx_sb = cpool.tile([128, NBLK * IDXC], dt.int16)
            nc.sync.dma_start(idx_sb[:], idx1.ap())
            wei_sb = cpool.tile([F, F], dt.float32)
            nc.sync.dma_start(wei_sb[:], wei.ap())
            id_sb = cpool.tile([128, 128], dt.float32)
            nc.sync.dma_start(id_sb[:], ident.ap())

            for b in range(NBLK):
                wg_t = spool.tile([128, F], dt.float32, tag="wg")
                nc.sync.dma_start(wg_t[:], wg_r[b])
                bg_t = spool.tile([128, 1], dt.float32, tag="bg")
                nc.sync.dma_start(bg_t[:], bg_r[b])
                nm_t = spool.tile([128, 1], dt.float32, tag="nm")
                nc.sync.dma_start(nm_t[:], nm_r[b])

                mail = mpool.tile([128, PAIRS_BLK], dt.float32)
                nc.gpsimd.dma_gather(
                    mail[:].rearrange("p (c f) -> p c f", f=F),
                    h32.ap(), idx_sb[:, b * IDXC:(b + 1) * IDXC],
                    PAIRS_BLK, PAIRS_BLK, F, single_packet=False,
                )
                m3 = mail[:].rearrange("p (d f) -> p d f", d=DEGREE)

                # logits[p, d] = sum_f mail[p, d, f] * wg[p, f]
                tmp = tpool.tile([128, PAIRS_BLK], dt.float32)
                wg_b = wg_t[:].unsqueeze(1).broadcast_to([128, DEGREE, F])
                nc.vector.tensor_tensor(
                    tmp[:].rearrange("p (d f) -> p d f", d=DEGREE),
                    m3, wg_b, AluOpType.mult,
                )
                lg = spool.tile([128, DEGREE], dt.float32, tag="lg")
                nc.vector.reduce_sum(
                    lg[:], tmp[:].rearrange("p (d f) -> p d f", d=DEGREE),
                    axis=mybir.AxisListType.X,
                )
                # mask = (logits + b_gate) > 0   (== round(sigmoid(.)))
                nc.vector.tensor_scalar(lg[:], lg[:], bg_t[:], None, AluOpType.add)
                mk = spool.tile([128, DEGREE], dt.float32, tag="mk")
                nc.vector.tensor_scalar(mk[:], lg[:], 0.0, None, AluOpType.is_gt)

                # h1 = sum_d mask * mail   (masked mult, then d-halving tree)
                mk_b = mk[:].unsqueeze(2).broadcast_to([128, DEGREE, F])
                nc.vector.tensor_tensor(
                    tmp[:].rearrange("p (d f) -> p d f", d=DEGREE),
                    m3, mk_b, AluOpType.mult,
                )
                h1_t = spool.tile([128, F], dt.float32, tag="h1")
                nc.vector.reduce_sum(
                    h1_t[:], tmp[:].rearrange("p (d f) -> p f d", d=DEGREE),
                    axis=mybir.AxisListType.X,
                )
                # h1 *= norm
                nc.vector.tensor_scalar(
                    h1_t[:], h1_t[:], nm_t[:], None, AluOpType.mult,
                )
                # h2 = h1 @ weight  (transpose h1 on PE, then matmul)
                h1T_ps = pspool.tile([128, 128], dt.float32, tag="tp")
                nc.tensor.transpose(h1T_ps[:], h1_t[:], id_sb[:])
                h1T = opool.tile([128, 128], dt.float32, tag="h1T")
                nc.vector.tensor_copy(h1T[:], h1T_ps[:])
                h2_ps = pspool.tile([128, F], dt.float32, tag="mm")
                nc.tensor.matmul(h2_ps[:], h1T[:], wei_sb[:], start=True, stop=True)
                h2_sb = opool.tile([128, F], dt.float32, tag="h2")
                nc.vector.tensor_copy(h2_sb[:], h2_ps[:])
                nc.sync.dma_start(h2o_r[b], h2_sb[:])
    nc.finalize()
    return nc


def _build_launch2():
    import concourse.bacc as bacc
    import concourse.mybir as mybir
    from concourse.mybir import AluOpType
    from concourse.tile import TileContext

    dt = mybir.dt
    nc = bacc.Bacc("TRN2", target_bir_lowering=False, debug=False)
    h2f = nc.dram_tensor("h2f", [N_NODES, F], dt.float32, kind="ExternalInput")
    idx2 = nc.dram_tensor("idx2", [128, NBLK * IDXC], dt.int16, kind="ExternalInput")
    nm = nc.dram_tensor("nm", [ROWS_PAD, 1], dt.float32, kind="ExternalInput")
    bia = nc.dram_tensor("bia", [128, F], dt.float32, kind="ExternalInput")
    h3o = nc.dram_tensor("h3o", [ROWS_PAD, F], dt.float32, kind="ExternalOutput")

    nm_r = nm.ap().rearrange("(b p) o -> b p o", p=128)
    h3o_r = h3o.ap().rearrange("(b p) f -> b p f", p=128)

    with TileContext(nc) as tc:
        with (
            tc.tile_pool(name="const", bufs=1) as cpool,
            tc.tile_pool(name="mail", bufs=4) as mpool,
            tc.tile_pool(name="small", bufs=4) as spool,
            tc.tile_pool(name="out", bufs=3) as opool,
        ):
            idx_sb = cpool.tile([128, NBLK * IDXC], dt.int16)
            nc.sync.dma_start(idx_sb[:], idx2.ap())
            bia_sb = cpool.tile([128, F], dt.float32)
            nc.sync.dma_start(bia_sb[:], bia.ap())

            for b in range(NBLK):
                nm_t = spool.tile([128, 1], dt.float32, tag="nm")
                nc.sync.dma_start(nm_t[:], nm_r[b])
                g = mpool.tile([128, PAIRS_BLK], dt.float32)
                nc.gpsimd.dma_gather(
                    g[:].rearrange("p (c f) -> p c f", f=F),
                    h2f.ap(), idx_sb[:, b * IDXC:(b + 1) * IDXC],
                    PAIRS_BLK, PAIRS_BLK, F, single_packet=False,
                )
                hs = spool.tile([128, F], dt.float32, tag="hs")
                nc.vector.reduce_sum(
                    hs[:], g[:].rearrange("p (d f) -> p f d", d=DEGREE),
                    axis=mybir.AxisListType.X,
                )
                nc.vector.tensor_scalar(
                    hs[:], hs[:], nm_t[:], None, AluOpType.mult,
                )
                h3 = opool.tile([128, F], dt.float32, tag="h3")
                nc.vector.tensor_tensor(h3[:], hs[:], bia_sb[:], AluOpType.add)
                nc.vector.tensor_scalar(h3[:], h3[:], 0.0, None, AluOpType.max)
                nc.sync.dma_start(h3o_r[b], h3[:])
    nc.finalize()
    return nc


def _get(name, builder):
    if name not in _cache:
        _cache[name] = builder()
    return _cache[name]


def kernel(h, neighbors, norm, W_gate, b_gate, weight, bias):
    from concourse import bass_utils

    h = np.asarray(h, dtype=np.float32)
    neighbors_in = np.asarray(neighbors)
    neighbors = neighbors_in.astype(np.int64)
    norm = np.asarray(norm, dtype=np.float32).reshape(N_NODES, 1)
    W_gate = np.asarray(W_gate, dtype=np.float32)
    b_gate = np.asarray(b_gate, dtype=np.float32).reshape(N_NODES, 1)
    weight = np.asarray(weight, dtype=np.float32)
    bias = np.asarray(bias, dtype=np.float32)

    pad = ROWS_PAD - ROWS_PER_CORE
    ident = np.eye(128, dtype=np.float32)
    bias_bc = np.broadcast_to(bias, (128, F)).copy()

    nc1 = _get("l1", _build_launch1)
    in_maps1 = []
    for c in range(N_CORES):
        s = slice(c * ROWS_PER_CORE, (c + 1) * ROWS_PER_CORE)
        nb = np.concatenate([neighbors[s], np.zeros((pad, DEGREE), np.int64)])
        in_maps1.append({
            "h32": h,
            "idx1": _gather_idx_for_core(nb),
            "wg": np.concatenate([W_gate[s], np.zeros((pad, F), np.float32)]),
            "bg": np.concatenate([b_gate[s], np.zeros((pad, 1), np.float32)]),
            "nm": np.concatenate([norm[s], np.zeros((pad, 1), np.float32)]),
            "wei": weight,
            "ident": ident,
        })
    import time as _time
    _t0 = _time.perf_counter()
    res1 = bass_utils.run_bass_kernel_spmd(nc1, in_maps1, core_ids=list(range(N_CORES)))
    _t1 = _time.perf_counter()
    kernel.launch_times = [_t1 - _t0]
    h2 = np.concatenate(
        [res1.results[c]["h2o"][:ROWS_PER_CORE] for c in range(N_CORES)]
    )

    nc2 = _get("l2", _build_launch2)
    in_maps2 = []
    for c in range(N_CORES):
        s = slice(c * ROWS_PER_CORE, (c + 1) * ROWS_PER_CORE)
        nb = np.concatenate([neighbors[s], np.zeros((pad, DEGREE), np.int64)])
        in_maps2.append({
            "h2f": h2,
            "idx2": _gather_idx_for_core(nb),
            "nm": np.concatenate([norm[s], np.zeros((pad, 1), np.float32)]),
            "bia": bias_bc,
        })
    _t0 = _time.perf_counter()
    res2 = bass_utils.run_bass_kernel_spmd(nc2, in_maps2, core_ids=list(range(N_CORES)))
    _t1 = _time.perf_counter()
    kernel.launch_times.append(_t1 - _t0)
    out = np.concatenate(
        [res2.results[c]["h3o"][:ROWS_PER_CORE] for c in range(N_CORES)]
    )
    return out.astype(np.float32)


# revision 9
# speedup vs baseline: 1.0657x; 1.0316x over previous
"""GCN layer (gather-gate-sum / dense / gather-sum) on 8 Trainium2 NeuronCores.

Sharding: nodes are split across the 8 cores (2500 rows each, padded to 2560).
The full node-feature table (h, then h2) stays replicated in each core's DRAM
and the neighbor gather is a DMAGather against it, so no halo exchange is
needed inside a launch.  The round-1 -> round-2 dependency (every core needs
every h2 row) is satisfied by a host-side gather between two launches.

Self-contained: shapes are hardcoded for N=20000, D=32, F=128, 8 cores.
"""
import os
import sys

sys.path.insert(0, "/opt/trn_rl_repo")

import numpy as np

N_NODES = 20000
DEGREE = 32
F = 128
N_CORES = 8
ROWS_PER_CORE = N_NODES // N_CORES          # 2500
NBLK = (ROWS_PER_CORE + 127) // 128         # 20 blocks of 128 rows
ROWS_PAD = NBLK * 128                       # 2560
PAIRS_BLK = 128 * DEGREE                    # 4096 gather indices per block
IDXC = PAIRS_BLK // 16                      # idx columns per block (wrapped in 16)

_cache = {}


def _wrap_idx(idx_flat):
    """Pack linear gather indices into the [128, n/16] int16 SBUF layout
    (index i lives at partition i%16, column i//16; replicated to 128)."""
    n = idx_flat.shape[0]
    assert n % 16 == 0
    w = np.zeros((16, n // 16), dtype=np.int16)
    w[np.arange(n) % 16, np.arange(n) // 16] = idx_flat.astype(np.int16)
    return np.tile(w, (8, 1))


def _gather_idx_for_core(nbrs_shard):
    """nbrs_shard: [ROWS_PAD, DEGREE] int.  Block b gathers its 128 rows'
    neighbors with linear order i = d*128 + p  (partition p = row-in-block,
    free block d = neighbor slot); wrapped layout [16, n/16] replicated x8."""
    lin = nbrs_shard.reshape(NBLK, 128, DEGREE).transpose(0, 2, 1).reshape(NBLK, PAIRS_BLK)
    w = lin.reshape(NBLK, IDXC, 16).transpose(0, 2, 1).astype(np.int16)  # [b, 16, IDXC]
    w = w.transpose(1, 0, 2).reshape(16, NBLK * IDXC)
    return np.tile(w, (8, 1))


def _build_launch1():
    import concourse.bacc as bacc
    import concourse.mybir as mybir
    from concourse.mybir import AluOpType
    from concourse.tile import TileContext

    dt = mybir.dt
    nc = bacc.Bacc("TRN2", target_bir_lowering=False, debug=False)
    h32 = nc.dram_tensor("h32", [N_NODES, F], dt.float32, kind="ExternalInput")
    idx1 = nc.dram_tensor("idx1", [128, NBLK * IDXC], dt.int16, kind="ExternalInput")
    wg = nc.dram_tensor("wg", [ROWS_PAD, F], dt.float32, kind="ExternalInput")
    bg = nc.dram_tensor("bg", [ROWS_PAD, 1], dt.float32, kind="ExternalInput")
    nm = nc.dram_tensor("nm", [ROWS_PAD, 1], dt.float32, kind="ExternalInput")
    wei = nc.dram_tensor("wei", [F, F], dt.float32, kind="ExternalInput")
    ident = nc.dram_tensor("ident", [128, 128], dt.float32, kind="ExternalInput")
    h2o = nc.dram_tensor("h2o", [ROWS_PAD, F], dt.float32, kind="ExternalOutput")

    wg_r = wg.ap().rearrange("(b p) f -> b p f", p=128)
    bg_r = bg.ap().rearrange("(b p) o -> b p o", p=128)
    nm_r = nm.ap().rearrange("(b p) o -> b p o", p=128)
    h2o_r = h2o.ap().rearrange("(b p) f -> b p f", p=128)

    with TileContext(nc) as tc:
        with (
            tc.tile_pool(name="const", bufs=1) as cpool,
            tc.tile_pool(name="mail", bufs=3) as mpool,
            tc.tile_pool(name="tmp", bufs=3) as tpool,
            tc.tile_pool(name="small", bufs=4) as spool,
            tc.tile_pool(name="out", bufs=3) as opool,
            tc.tile_pool(name="ps", bufs=4, space="PSUM") as pspool,
        ):
            idx_sb = cpool.tile([128, NBLK * IDXC], dt.int16)
            nc.sync.dma_start(idx_sb[:], idx1.ap())
            wei_sb = cpool.tile([F, F], dt.float32)
            nc.sync.dma_start(wei_sb[:], wei.ap())
            id_sb = cpool.tile([128, 128], dt.float32)
            nc.sync.dma_start(id_sb[:], ident.ap())

            for b in range(NBLK):
                wg_t = spool.tile([128, F], dt.float32, tag="wg")
                nc.sync.dma_start(wg_t[:], wg_r[b])
                bg_t = spool.tile([128, 1], dt.float32, tag="bg")
                nc.sync.dma_start(bg_t[:], bg_r[b])
                nm_t = spool.tile([128, 1], dt.float32, tag="nm")
                nc.sync.dma_start(nm_t[:], nm_r[b])

                mail = mpool.tile([128, PAIRS_BLK], dt.float32)
                nc.gpsimd.dma_gather(
                    mail[:].rearrange("p (c f) -> p c f", f=F),
                    h32.ap(), idx_sb[:, b * IDXC:(b + 1) * IDXC],
                    PAIRS_BLK, PAIRS_BLK, F, single_packet=False,
                )
                m3 = mail[:].rearrange("p (d f) -> p d f", d=DEGREE)

                # logits[p, d] = sum_f mail[p, d, f] * wg[p, f]
                tmp = tpool.tile([128, PAIRS_BLK], dt.float32)
                wg_b = wg_t[:].unsqueeze(1).broadcast_to([128, DEGREE, F])
                nc.vector.tensor_tensor(
                    tmp[:].rearrange("p (d f) -> p d f", d=DEGREE),
                    m3, wg_b, AluOpType.mult,
                )
                lg = spool.tile([128, DEGREE], dt.float32, tag="lg")
                nc.vector.reduce_sum(
                    lg[:], tmp[:].rearrange("p (d f) -> p d f", d=DEGREE),
                    axis=mybir.AxisListType.X,
                )
                # mask = (logits + b_gate) > 0   (== round(sigmoid(.)))
                nc.vector.tensor_scalar(lg[:], lg[:], bg_t[:], None, AluOpType.add)
                mk = spool.tile([128, DEGREE], dt.float32, tag="mk")
                nc.vector.tensor_scalar(mk[:], lg[:], 0.0, None, AluOpType.is_gt)

                # h1 = sum_d mask * mail   (masked mult, then d-halving tree)
                mk_b = mk[:].unsqueeze(2).broadcast_to([128, DEGREE, F])
                nc.gpsimd.tensor_tensor(
                    tmp[:].rearrange("p (d f) -> p d f", d=DEGREE),
                    m3, mk_b, AluOpType.mult,
                )
                h1_t = spool.tile([128, F], dt.float32, tag="h1")
                nc.vector.reduce_sum(
                    h1_t[:], tmp[:].rearrange("p (d f) -> p f d", d=DEGREE),
                    axis=mybir.AxisListType.X,
                )
                # h1 *= norm
                nc.vector.tensor_scalar(
                    h1_t[:], h1_t[:], nm_t[:], None, AluOpType.mult,
                )
                # h2 = h1 @ weight  (transpose h1 on PE, then matmul)
                h1T_ps = pspool.tile([128, 128], dt.float32, tag="tp")
                nc.tensor.transpose(h1T_ps[:], h1_t[:], id_sb[:])
                h1T = opool.tile([128, 128], dt.float32, tag="h1T")
                nc.vector.tensor_copy(h1T[:], h1T_ps[:])
                h2_ps = pspool.tile([128, F], dt.float32, tag="mm")
                nc.tensor.matmul(h2_ps[:], h1T[:], wei_sb[:], start=True, stop=True)
                h2_sb = opool.tile([128, F], dt.float32, tag="h2")
                nc.vector.tensor_copy(h2_sb[:], h2_ps[:])
                nc.sync.dma_start(h2o_r[b], h2_sb[:])
    nc.finalize()
    return nc


def _build_launch2():
    import concourse.bacc as bacc
    import concourse.mybir as mybir
    from concourse.mybir import AluOpType
    from concourse.tile import TileContext

    dt = mybir.dt
    nc = bacc.Bacc("TRN2", target_bir_lowering=False, debug=False)
    h2f = nc.dram_tensor("h2f", [N_NODES, F], dt.float32, kind="ExternalInput")
    idx2 = nc.dram_tensor("idx2", [128, NBLK * IDXC], dt.int16, kind="ExternalInput")
    nm = nc.dram_tensor("nm", [ROWS_PAD, 1], dt.float32, kind="ExternalInput")
    bia = nc.dram_tensor("bia", [128, F], dt.float32, kind="ExternalInput")
    h3o = nc.dram_tensor("h3o", [ROWS_PAD, F], dt.float32, kind="ExternalOutput")

    nm_r = nm.ap().rearrange("(b p) o -> b p o", p=128)
    h3o_r = h3o.ap().rearrange("(b p) f -> b p f", p=128)

    with TileContext(nc) as tc:
        with (
            tc.tile_pool(name="const", bufs=1) as cpool,
            tc.tile_pool(name="mail", bufs=4) as mpool,
            tc.tile_pool(name="small", bufs=4) as spool,
            tc.tile_pool(name="out", bufs=3) as opool,
        ):
            idx_sb = cpool.tile([128, NBLK * IDXC], dt.int16)
            nc.sync.dma_start(idx_sb[:], idx2.ap())
            bia_sb = cpool.tile([128, F], dt.float32)
            nc.sync.dma_start(bia_sb[:], bia.ap())

            for b in range(NBLK):
                nm_t = spool.tile([128, 1], dt.float32, tag="nm")
                nc.sync.dma_start(nm_t[:], nm_r[b])
                g = mpool.tile([128, PAIRS_BLK], dt.float32)
                nc.gpsimd.dma_gather(
                    g[:].rearrange("p (c f) -> p c f", f=F),
                    h2f.ap(), idx_sb[:, b * IDXC:(b + 1) * IDXC],
                    PAIRS_BLK, PAIRS_BLK, F, single_packet=False,
                )
                hs = spool.tile([128, F], dt.float32, tag="hs")
                nc.vector.reduce_sum(
                    hs[:], g[:].rearrange("p (d f) -> p f d", d=DEGREE),
                    axis=mybir.AxisListType.X,
                )
                nc.vector.tensor_scalar(
                    hs[:], hs[:], nm_t[:], None, AluOpType.mult,
                )
                h3 = opool.tile([128, F], dt.float32, tag="h3")
                nc.vector.tensor_tensor(h3[:], hs[:], bia_sb[:], AluOpType.add)
                nc.vector.tensor_scalar(h3[:], h3[:], 0.0, None, AluOpType.max)
                nc.sync.dma_start(h3o_r[b], h3[:])
    nc.finalize()
    return nc


def _get(name, builder):
    if name not in _cache:
        _cache[name] = builder()
    return _cache[name]


def kernel(h, neighbors, norm, W_gate, b_gate, weight, bias):
    from concourse import bass_utils

    h = np.asarray(h, dtype=np.float32)
    neighbors_in = np.asarray(neighbors)
    neighbors = neighbors_in.astype(np.int64)
    norm = np.asarray(norm, dtype=np.float32).reshape(N_NODES, 1)
    W_gate = np.asarray(W_gate, dtype=np.float32)
    b_gate = np.asarray(b_gate, dtype=np.float32).reshape(N_NODES, 1)
    weight = np.asarray(weight, dtype=np.float32)
    bias = np.asarray(bias, dtype=np.float32)

    pad = ROWS_PAD - ROWS_PER_CORE
    ident = np.eye(128, dtype=np.float32)
    bias_bc = np.broadcast_to(bias, (128, F)).copy()

    nc1 = _get("l1", _build_launch1)
    in_maps1 = []
    for c in range(N_CORES):
        s = slice(c * ROWS_PER_CORE, (c + 1) * ROWS_PER_CORE)
        nb = np.concatenate([neighbors[s], np.zeros((pad, DEGREE), np.int64)])
        in_maps1.append({
            "h32": h,
            "idx1": _gather_idx_for_core(nb),
            "wg": np.concatenate([W_gate[s], np.zeros((pad, F), np.float32)]),
            "bg": np.concatenate([b_gate[s], np.zeros((pad, 1), np.float32)]),
            "nm": np.concatenate([norm[s], np.zeros((pad, 1), np.float32)]),
            "wei": weight,
            "ident": ident,
        })
    import time as _time
    _t0 = _time.perf_counter()
    res1 = bass_utils.run_bass_kernel_spmd(nc1, in_maps1, core_ids=list(range(N_CORES)))
    _t1 = _time.perf_counter()
    kernel.launch_times = [_t1 - _t0]
    h2 = np.concatenate(
        [res1.results[c]["h2o"][:ROWS_PER_CORE] for c in range(N_CORES)]
    )

    nc2 = _get("l2", _build_launch2)
    in_maps2 = []
    for c in range(N_CORES):
        s = slice(c * ROWS_PER_CORE, (c + 1) * ROWS_PER_CORE)
        nb = np.concatenate([neighbors[s], np.zeros((pad, DEGREE), np.int64)])
        in_maps2.append({
            "h2f": h2,
            "idx2": _gather_idx_for_core(nb),
            "nm": np.concatenate([norm[s], np.zeros((pad, 1), np.float32)]),
            "bia": bias_bc,
        })
    _t0 = _time.perf_counter()
    res2 = bass_utils.run_bass_kernel_spmd(nc2, in_maps2, core_ids=list(range(N_CORES)))
    _t1 = _time.perf_counter()
    kernel.launch_times.append(_t1 - _t0)
    out = np.concatenate(
        [res2.results[c]["h3o"][:ROWS_PER_CORE] for c in range(N_CORES)]
    )
    return out.astype(np.float32)
